# revision 9
# baseline (speedup 1.0000x reference)
"""MoE (top-1, capacity_factor=1) Trainium2 Bass kernel, expert-parallel over
8 NeuronCores. Self-contained: imports only numpy + concourse (/opt).

Per-core k (expert k resident):
  gate GEMM (fp32 exact) on its 1/8 token shard -> AllGather logits ->
  replicated routing (one-hot, global cumsum via triangular matmuls,
  capacity mask) -> slot->token table via gpsimd sparse_gather ->
  indirect row-gather of x + PE transposes -> dispT [m, c] ->
  GEMM1 (fp32r, w1 stationary) -> hT [h, c] with fused ReLU+b1 ->
  GEMM2 (fp32r, w2 stationary, host pre-tiled) -> eoT [m, c],
  + b2 + gate scaling -> EOT output.
Host: scatter rows eo = EOT.T back by token id (A table), empties G==0.
"""
import sys

sys.path.insert(0, '/opt/trn_rl_repo')

import numpy as np
import concourse.bass as bass
import concourse.tile as tile
import concourse.mybir as mybir
from concourse import bacc
from concourse.bass_utils import run_bass_kernel_spmd
from concourse.masks import make_upper_triangular, make_identity

N_CORES = 8
B, SQ, M, E, H = 4, 2048, 1024, 8, 4096
S = B * SQ            # 8192 tokens
C = S // E            # 1024 capacity
NT = S // 128         # 64 token tiles
NTS = NT // N_CORES   # 8 token tiles per core shard
CBLK = 512            # c-block for GEMM1/GEMM2
NCB = C // CBLK       # 2 c-blocks
BIG = 2_000_000.0
F_IN = (S + C) // 16  # 576  sparse_gather input free size
F_OUT = C // 16       # 64

f32 = mybir.dt.float32
f32r = mybir.dt.float32r
f16 = mybir.dt.float16
i32 = mybir.dt.int32
u32 = mybir.dt.uint32
AX = mybir.AxisListType
OP = mybir.AluOpType
ACTF = mybir.ActivationFunctionType


def _split_multi_waits(nc):
    """This walrus build accepts at most ONE sync-wait per instruction.
    Split extras into same-engine NOPs inserted just before."""
    from concourse.mybir import SyncInfo
    n = 0
    for bb in list(nc.main_func.blocks):
        insts = bb.instructions  # live shared list
        for ins in list(insts):
            si = ins.sync_info
            if si is None or len(si.on_wait) <= 1:
                continue
            waits = list(si.on_wait)
            idx = insts.index(ins)
            for j, w in enumerate(waits[:-1]):
                nop = nc.engines[ins.engine].nop(nofuse=True, hint="waitsplit")
                ni = nop.ins
                cur = nc.cur_bb.bb.instructions
                if ni in cur:
                    cur.remove(ni)
                ni.sync_info = SyncInfo(on_wait=[w], on_update=[])
                insts.insert(idx + j, ni)
                n += 1
            ins.sync_info = SyncInfo(on_wait=[waits[-1]], on_update=si.on_update)
    return n


def r3(ap, e=E):
    return ap.rearrange("p (t e) -> p t e", e=e)


def build():
    nc = bacc.Bacc("TRN2", target_bir_lowering=False, debug=False,
                   num_devices=N_CORES)

    x_ap = nc.dram_tensor("x", [S, M], f32, kind="ExternalInput").ap()
    xts_ap = nc.dram_tensor("xts", [M, S // N_CORES], f32, kind="ExternalInput").ap()
    wg_ap = nc.dram_tensor("wg", [M, E], f32, kind="ExternalInput").ap()
    w1_ap = nc.dram_tensor("w1", [M, H], f16, kind="ExternalInput").ap()
    b1c_ap = nc.dram_tensor("b1c", [128, H // 128], f32, kind="ExternalInput").ap()
    w2t_ap = nc.dram_tensor("w2t", [M // 128, 128, H], f16, kind="ExternalInput").ap()
    b2c_ap = nc.dram_tensor("b2c", [128, M // 128], f32, kind="ExternalInput").ap()
    eid_ap = nc.dram_tensor("eid", [128, 1], f32, kind="ExternalInput").ap()

    eot_ap = nc.dram_tensor("EOT", [M, C], f32, kind="ExternalOutput").ap()
    a_ap = nc.dram_tensor("A", [C, 1], i32, kind="ExternalOutput").ap()
    g_ap = nc.dram_tensor("G", [C, 1], f32, kind="ExternalOutput").ap()

    with tile.TileContext(nc) as tc:
        with tc.tile_pool(name="consts", bufs=1) as cp, \
             tc.tile_pool(name="rt", bufs=1) as rt, \
             tc.tile_pool(name="dram", bufs=1, space="DRAM") as dram:

            # ---------------- constants ----------------
            TRI = cp.tile([128, 128], f32)       # TRI[q,p]=1 iff q<=p
            make_upper_triangular(nc, TRI[:], val=1.0, diag=True)
            TRIS = cp.tile([64, 64], f32)        # strict upper
            make_upper_triangular(nc, TRIS[:], val=1.0, diag=False)
            IDN = cp.tile([128, 128], f32)
            make_identity(nc, IDN[:])
            ones_col = cp.tile([128, 1], f32)
            nc.vector.memset(ones_col[:], 1.0)
            ones_row = cp.tile([1, 128], f32)
            nc.vector.memset(ones_row[:], 1.0)
            iota_e = cp.tile([128, NT * E], f32)
            nc.gpsimd.iota(r3(iota_e[:]), pattern=[[0, NT], [1, E]], base=0,
                           channel_multiplier=0,
                           allow_small_or_imprecise_dtypes=True)
            tokf = cp.tile([128, NT], f32)
            nc.gpsimd.iota(tokf[:], pattern=[[128, NT]], base=0,
                           channel_multiplier=1,
                           allow_small_or_imprecise_dtypes=True)
            eid_sb = cp.tile([128, 1], f32)
            nc.sync.dma_start(eid_sb[:], eid_ap[:])
            b1c = cp.tile([128, H // 128], f32)
            nc.sync.dma_start(b1c[:], b1c_ap[:])
            b2c = cp.tile([128, M // 128], f32)
            nc.sync.dma_start(b2c[:], b2c_ap[:])

            # ---------------- phase 1: gate GEMM (fp32) ----------------
            wg_sb = cp.tile([128, (M // 128) * E], f32)   # [p, (mc, e)]
            nc.sync.dma_start(r3(wg_sb[:]),
                              wg_ap.rearrange("(mc p) e -> p mc e", p=128))
            Lg_sb = rt.tile([128, NTS * E], f32)          # own shard logits
            Lg_in = dram.tile([S // N_CORES, E], f32)
            Lg_all = dram.tile([S, E], f32, addr_space="Shared")

            with tc.tile_pool(name="xts", bufs=1) as xp, \
                 tc.tile_pool(name="psg_p", bufs=1, space="PSUM") as psg_p:
                xts_strips = []
                for mc in range(M // 128):
                    st = xp.tile([128, S // N_CORES], f32, tag=f"xts{mc}")
                    nc.sync.dma_start(st[:], xts_ap[mc * 128:(mc + 1) * 128, :])
                    xts_strips.append(st)
                for t in range(NTS):
                    psg = psg_p.tile([128, E], f32, tag="psg")
                    for mc in range(M // 128):
                        nc.tensor.matmul(
                            psg[:], xts_strips[mc][:, t * 128:(t + 1) * 128],
                            wg_sb[:, mc * E:(mc + 1) * E],
                            start=(mc == 0), stop=(mc == M // 128 - 1))
                    nc.vector.tensor_copy(Lg_sb[:, t * E:(t + 1) * E], psg[:])

            nc.sync.dma_start(Lg_in[:].rearrange("(t p) e -> p t e", p=128),
                              r3(Lg_sb[:]))
            nc.gpsimd.collective_compute(
                "AllGather", OP.bypass,
                replica_groups=[list(range(N_CORES))],
                ins=[Lg_in[:]], outs=[Lg_all[:]])

            bigp_cm = tc.tile_pool(name="big", bufs=1)
            bigp = bigp_cm.__enter__()
            wk_cm = tc.tile_pool(name="wk", bufs=2)
            wk = wk_cm.__enter__()

            # PE warm-up machinery: dummy matmuls on constant data keep the
            # HAM activity monitor at full clock through the DMA-heavy
            # prologue so the FFN GEMMs start warm.
            warm_cm = tc.tile_pool(name="warm_ps", bufs=1, space="PSUM")
            warm_ps = warm_cm.__enter__()
            warm_out = warm_ps.tile([128, 256], f32, tag="warm")
            warm_in = cp.tile([128, 256], mybir.dt.bfloat16)
            nc.vector.memset(warm_in[:], 1.0)
            warm_w = cp.tile([128, 128], mybir.dt.bfloat16)
            nc.vector.memset(warm_w[:], 1.0)

            def pe_warm(n):
                for _ in range(n):
                    nc.tensor.matmul(warm_out[:], warm_w[:], warm_in[:],
                                     start=True, stop=True,
                                     skip_group_check=True)

            # ---------------- phase 2: routing (replicated) ----------------
            L_all = rt.tile([128, NT * E], f32)
            for lc in range(8):
                tchunk = NT // 8
                nc.sync.dma_start(
                    r3(L_all[:, lc * tchunk * E:(lc + 1) * tchunk * E]),
                    Lg_all[lc * tchunk * 128:(lc + 1) * tchunk * 128, :]
                    .rearrange("(t p) e -> p t e", p=128))
            L3 = r3(L_all[:])

            lmax = rt.tile([128, NT], f32)
            nc.vector.tensor_reduce(lmax[:], L3, axis=AX.X, op=OP.max)
            lmax_b = lmax[:].rearrange("p (t o) -> p t o", o=1) \
                            .to_broadcast([128, NT, E])
            dd = rt.tile([128, NT * E], f32)
            nc.vector.tensor_tensor(out=r3(dd[:]), in0=L3, in1=lmax_b,
                                    op=OP.subtract)
            expd = rt.tile([128, NT * E], f32)
            nc.scalar.activation(expd[:], dd[:], ACTF.Exp)
            ssum = rt.tile([128, NT], f32)
            nc.vector.tensor_reduce(ssum[:], r3(expd[:]), axis=AX.X, op=OP.add)
            gv = rt.tile([128, NT], f32)
            nc.vector.reciprocal(gv[:], ssum[:])

            oh = rt.tile([128, NT * E], f32)
            nc.vector.tensor_tensor(out=r3(oh[:]), in0=L3, in1=lmax_b,
                                    op=OP.is_equal)
            tmp = rt.tile([128, NT * E], f32)
            nc.vector.tensor_mul(tmp[:], oh[:], iota_e[:])
            eidx = rt.tile([128, NT], f32)
            nc.vector.tensor_reduce(eidx[:], r3(tmp[:]), axis=AX.X, op=OP.add)

            ps_rt_cm = tc.tile_pool(name="ps_rt", bufs=1, space="PSUM")
            ps_rt = ps_rt_cm.__enter__()
            pos_ps = ps_rt.tile([128, NT * E], f32, tag="pos")
            nc.tensor.matmul(pos_ps[:], TRI[:], oh[:], start=True, stop=False)
            cnt_ps = ps_rt.tile([1, NT * E], f32, tag="cnt")
            nc.tensor.matmul(cnt_ps[:], ones_col[:], oh[:], start=True, stop=True)
            cnt_sb = rt.tile([1, NT * E], f32)
            nc.vector.tensor_copy(cnt_sb[:], cnt_ps[:])
            cntd = dram.tile([64, 8], f32)
            nc.sync.dma_start(cntd[:].rearrange("t e -> (t e)"), cnt_sb[0:1, :])
            cnt64 = rt.tile([64, 8], f32)
            nc.sync.dma_start(cnt64[:], cntd[:])
            offs_ps = ps_rt.tile([64, 8], f32, tag="offs")
            nc.tensor.matmul(offs_ps[:], TRIS[:], cnt64[:], start=True, stop=True)
            offs_sb = rt.tile([64, 8], f32)
            nc.vector.tensor_copy(offs_sb[:], offs_ps[:])
            offsd = dram.tile([64, 8], f32)
            nc.sync.dma_start(offsd[:], offs_sb[:])
            offs_flat = rt.tile([1, NT * E], f32)
            nc.sync.dma_start(offs_flat[0:1, :], offsd[:].rearrange("t e -> (t e)"))
            nc.tensor.matmul(pos_ps[:], ones_row[:], offs_flat[:],
                             start=False, stop=True)

            pe_warm(8)
            pm = rt.tile([128, NT * E], f32)
            nc.vector.tensor_mul(pm[:], pos_ps[:], oh[:])
            pos_tok = rt.tile([128, NT], f32)
            nc.vector.tensor_reduce(pos_tok[:], r3(pm[:]), axis=AX.X, op=OP.add)
            nc.vector.tensor_scalar_add(pos_tok[:], pos_tok[:], -1.0)
            ps_rt_cm.__exit__(None, None, None)

            keep = rt.tile([128, NT], f32)
            nc.vector.tensor_scalar(out=keep[:], in0=pos_tok[:],
                                    scalar1=float(C), scalar2=None, op0=OP.is_lt)
            mine = rt.tile([128, NT], f32)
            nc.vector.tensor_scalar(out=mine[:], in0=eidx[:],
                                    scalar1=eid_sb[:, 0:1], scalar2=None,
                                    op0=OP.is_equal)
            sel = rt.tile([128, NT], f32)
            nc.vector.tensor_mul(sel[:], mine[:], keep[:])
            gvk = rt.tile([128, NT], f32)
            nc.vector.tensor_mul(gvk[:], gv[:], keep[:])

            # masked payloads: value if sel else -1; cols NT..NT+7 stay 0
            # (always-kept sentinels completing the capacity C region)
            XCOL = NT + C // 128
            mtok = rt.tile([128, XCOL], f32)
            nc.vector.memset(mtok[:, NT:], 0.0)
            nc.vector.tensor_scalar_add(mtok[:, :NT], tokf[:], 1.0)
            nc.vector.tensor_mul(mtok[:, :NT], mtok[:, :NT], sel[:])
            nc.vector.tensor_scalar_add(mtok[:, :NT], mtok[:, :NT], -1.0)
            mgv = rt.tile([128, XCOL], f32)
            nc.vector.memset(mgv[:, NT:], 0.0)
            nc.vector.tensor_scalar_add(mgv[:, :NT], gvk[:], 1.0)
            nc.vector.tensor_mul(mgv[:, :NT], mgv[:, :NT], sel[:])
            nc.vector.tensor_scalar_add(mgv[:, :NT], mgv[:, :NT], -1.0)

            # ------ sparse_gather compaction: slot -> (tokid, gate) ------
            Vd1 = dram.tile([S + C], f32)
            Vd2 = dram.tile([S + C], f32)
            nc.sync.dma_start(Vd1[:].rearrange("(t p) -> p t", p=128), mtok[:])
            nc.sync.dma_start(Vd2[:].rearrange("(t p) -> p t", p=128), mgv[:])
            pe_warm(8)
            V1 = rt.tile([16, F_IN], f32)
            nc.sync.dma_start(V1[:], Vd1[:].rearrange("(f q) -> q f", q=16))
            V2 = rt.tile([16, F_IN], f32)
            nc.sync.dma_start(V2[:], Vd2[:].rearrange("(f q) -> q f", q=16))
            SG1 = rt.tile([16, F_OUT], f32)
            NF1 = rt.tile([1, 1], u32)
            nc.gpsimd.sparse_gather(SG1[:], V1[:], num_found=NF1[:])
            SG2 = rt.tile([16, F_OUT], f32)
            NF2 = rt.tile([1, 1], u32)
            nc.gpsimd.sparse_gather(SG2[:], V2[:], num_found=NF2[:])

            A_f = dram.tile([C], f32)
            nc.sync.dma_start(A_f[:].rearrange("(f q) -> q f", q=16), SG1[:])
            G_f = dram.tile([C], f32)
            nc.sync.dma_start(G_f[:].rearrange("(f q) -> q f", q=16), SG2[:])

            pe_warm(8)
            a_slot_f = rt.tile([128, C // 128], f32)
            nc.sync.dma_start(a_slot_f[:], A_f[:].rearrange("(s p) -> p s", p=128))
            a_i = rt.tile([128, C // 128], i32)
            nc.vector.tensor_copy(a_i[:], a_slot_f[:])
            nc.sync.dma_start(a_ap.rearrange("(s p) o -> p (s o)", p=128), a_i[:])
            g_sb = rt.tile([128, C // 128], f32)
            nc.sync.dma_start(g_sb[:], G_f[:].rearrange("(s p) -> p s", p=128))
            nc.sync.dma_start(g_ap.rearrange("(s p) o -> p (s o)", p=128), g_sb[:])

            # ---------------- phase 3: dispatch gather + transpose --------
            pe_warm(8)
            dispT = bigp.tile([128, (M // 128) * C], f16)  # [p, (mc, c)] 32KB/p
            ps_tr_cm = tc.tile_pool(name="ps_tr", bufs=2, space="PSUM")
            ps_tr = ps_tr_cm.__enter__()
            for sg in range(C // 128):
                dg = wk.tile([128, M], f32, tag="dg")
                nc.gpsimd.indirect_dma_start(
                    out=dg[:], out_offset=None, in_=x_ap,
                    in_offset=bass.IndirectOffsetOnAxis(
                        ap=a_i[:, sg:sg + 1], axis=0))
                for mc in range(M // 128):
                    tp = ps_tr.tile([128, 128], f32, tag="tr")
                    nc.tensor.transpose(tp[:], dg[:, mc * 128:(mc + 1) * 128],
                                        IDN[:])
                    nc.vector.tensor_copy(
                        dispT[:, mc * C + sg * 128: mc * C + (sg + 1) * 128],
                        tp[:])

            # ---------------- phases 4+5: expert FFN ----------------
            ps_tr_cm.__exit__(None, None, None)
            warm_cm.__exit__(None, None, None)
            ps_ffn_cm = tc.tile_pool(name="ps_ffn", bufs=1, space="PSUM")
            ps_ffn = ps_ffn_cm.__enter__()
            hT = bigp.tile([128, (H // 128) * CBLK], f16)  # [p, (ht, c)] 64KB/p
            for cb in range(NCB):
                # GEMM1: hT[h, c] = relu(w1.T @ dispT + b1)
                for htb in range(H // CBLK):              # 8 blocks of 4 ht
                    w1s = []
                    for mc in range(M // 128):
                        w1sl = wk.tile([128, CBLK], f16, tag="w1sl", bufs=6)
                        nc.sync.dma_start(
                            w1sl[:],
                            w1_ap[mc * 128:(mc + 1) * 128,
                                  htb * CBLK:(htb + 1) * CBLK])
                        w1s.append(w1sl)
                    for hi in range(CBLK // 128):         # 4 ht per block
                        ht_i = htb * (CBLK // 128) + hi
                        ps1 = ps_ffn.tile([128, CBLK], f32, tag="g1", bufs=4)
                        for mc in range(M // 128):
                            nc.tensor.matmul(
                                ps1[:],
                                w1s[mc][:, hi * 128:(hi + 1) * 128],
                                dispT[:, mc * C + cb * CBLK:
                                      mc * C + (cb + 1) * CBLK],
                                start=(mc == 0), stop=(mc == M // 128 - 1))
                        nc.scalar.activation(
                            hT[:, ht_i * CBLK:(ht_i + 1) * CBLK], ps1[:],
                            ACTF.Relu, bias=b1c[:, ht_i:ht_i + 1], scale=1.0)

                # gate row broadcast for this c-block
                g_row = wk.tile([1, CBLK], f32, tag="grow")
                nc.sync.dma_start(g_row[0:1, :],
                                  G_f[cb * CBLK:(cb + 1) * CBLK])
                gb_ps = ps_ffn.tile([128, CBLK], f32, tag="g1", bufs=4)
                nc.tensor.matmul(gb_ps[:], ones_row[:], g_row[:],
                                 start=True, stop=True)
                g_bc = wk.tile([128, CBLK], f32, tag="gbc")
                nc.vector.tensor_copy(g_bc[:], gb_ps[:])

                # GEMM2: eoT[m, c] = w2.T @ hT ; then (+b2) * gate
                for mt in range(M // 128):
                    w2ts = wk.tile([128, H], f16, tag="w2ts", bufs=2)
                    nc.sync.dma_start(
                        w2ts[:], w2t_ap[mt].rearrange("p (hc m) -> p hc m", m=128))
                    ps2 = ps_ffn.tile([128, CBLK], f32, tag="g2", bufs=2)
                    for hc in range(H // 128):
                        nc.tensor.matmul(
                            ps2[:], w2ts[:, hc * 128:(hc + 1) * 128],
                            hT[:, hc * CBLK:(hc + 1) * CBLK],
                            start=(hc == 0), stop=(hc == H // 128 - 1))
                    eo_sb = wk.tile([128, CBLK], f32, tag="eo")
                    nc.vector.tensor_scalar(out=eo_sb[:], in0=ps2[:],
                                            scalar1=b2c[:, mt:mt + 1],
                                            scalar2=None, op0=OP.add)
                    nc.vector.tensor_mul(eo_sb[:], eo_sb[:], g_bc[:])
                    nc.sync.dma_start(
                        eot_ap[mt * 128:(mt + 1) * 128,
                               cb * CBLK:(cb + 1) * CBLK], eo_sb[:])

            ps_ffn_cm.__exit__(None, None, None)
            wk_cm.__exit__(None, None, None)
            bigp_cm.__exit__(None, None, None)

    nc.compile()
    _split_multi_waits(nc)
    return nc


_NC_CACHE = None


def _get_nc():
    global _NC_CACHE
    if _NC_CACHE is None:
        _NC_CACHE = build()
    return _NC_CACHE


def _make_in_maps(x, wg, w1, b1, w2, b2):
    x2 = np.ascontiguousarray(np.asarray(x, np.float32).reshape(S, M))
    wg = np.ascontiguousarray(np.asarray(wg, np.float32))
    w1 = np.asarray(w1, np.float32)
    b1 = np.asarray(b1, np.float32)
    w2 = np.asarray(w2, np.float32)
    b2 = np.asarray(b2, np.float32)
    in_maps = []
    for k in range(N_CORES):
        shard = x2[k * (S // N_CORES):(k + 1) * (S // N_CORES)]
        xts = np.ascontiguousarray(shard.T)                    # [M, S/8]
        w1k = np.ascontiguousarray(w1[k]).astype(np.float16)   # [M, H]
        b1ck = np.ascontiguousarray(b1[k].reshape(H // 128, 128).T)
        w2k = w2[k]                                            # [H, M]
        w2t = np.ascontiguousarray(
            w2k.reshape(H // 128, 128, M // 128, 128).transpose(2, 1, 0, 3)
        ).astype(np.float16)
        b2ck = np.ascontiguousarray(b2[k].reshape(M // 128, 128).T)
        eid = np.full((128, 1), k, np.float32)
        in_maps.append({
            "x": x2, "xts": xts, "wg": wg, "w1": w1k, "b1c": b1ck,
            "w2t": w2t, "b2c": b2ck, "eid": eid,
        })
    return in_maps


def run_cores(x, wg, w1, b1, w2, b2, trace=False, tmpdir=None):
    nc = _get_nc()
    in_maps = _make_in_maps(x, wg, w1, b1, w2, b2)
    return run_bass_kernel_spmd(nc, in_maps, list(range(N_CORES)), trace=trace,
                                tmpdir=tmpdir)


def combine(results):
    out = np.zeros((S, M), np.float32)
    for k in range(N_CORES):
        r = results[k]
        eo = np.ascontiguousarray(r["EOT"].T)    # [C, M]
        A = r["A"][:, 0].astype(np.int64)
        G = r["G"][:, 0]
        valid = G > 0
        out[A[valid]] = eo[valid]
    return out.reshape(B, SQ, M)


def kernel(x, wg, w1, b1, w2, b2):
    res = run_cores(x, wg, w1, b1, w2, b2, trace=False)
    return combine(res.results)


# revision 12
# speedup vs baseline: 1.1452x; 1.1452x over previous
"""MoE (top-1, capacity_factor=1) Trainium2 Bass kernel, expert-parallel over
8 NeuronCores. Self-contained: imports only numpy + concourse (/opt).

Per-core k (expert k resident):
  gate GEMM (fp32 exact) on its 1/8 token shard -> AllGather logits ->
  replicated routing (one-hot, global cumsum via triangular matmuls,
  capacity mask) -> slot->token table via gpsimd sparse_gather ->
  indirect row-gather of x + PE transposes -> dispT [m, c] ->
  GEMM1 (fp32r, w1 stationary) -> hT [h, c] with fused ReLU+b1 ->
  GEMM2 (fp32r, w2 stationary, host pre-tiled) -> eoT [m, c],
  + b2 + gate scaling -> EOT output.
Host: scatter rows eo = EOT.T back by token id (A table), empties G==0.
"""
import sys

sys.path.insert(0, '/opt/trn_rl_repo')

import numpy as np
import concourse.bass as bass
import concourse.tile as tile
import concourse.mybir as mybir
from concourse import bacc
from concourse.bass_utils import run_bass_kernel_spmd
from concourse.masks import make_upper_triangular, make_identity

N_CORES = 8
B, SQ, M, E, H = 4, 2048, 1024, 8, 4096
S = B * SQ            # 8192 tokens
C = S // E            # 1024 capacity
NT = S // 128         # 64 token tiles
NTS = NT // N_CORES   # 8 token tiles per core shard
CBLK = 512            # c-block for GEMM1/GEMM2
NCB = C // CBLK       # 2 c-blocks
BIG = 2_000_000.0
F_IN = (S + C) // 16  # 576  sparse_gather input free size
F_OUT = C // 16       # 64

f32 = mybir.dt.float32
f32r = mybir.dt.float32r
f16 = mybir.dt.float16
i32 = mybir.dt.int32
u32 = mybir.dt.uint32
AX = mybir.AxisListType
OP = mybir.AluOpType
ACTF = mybir.ActivationFunctionType


def _split_multi_waits(nc):
    """This walrus build accepts at most ONE sync-wait per instruction.
    Split extras into same-engine NOPs inserted just before."""
    from concourse.mybir import SyncInfo
    n = 0
    for bb in list(nc.main_func.blocks):
        insts = bb.instructions  # live shared list
        for ins in list(insts):
            si = ins.sync_info
            if si is None or len(si.on_wait) <= 1:
                continue
            waits = list(si.on_wait)
            idx = insts.index(ins)
            for j, w in enumerate(waits[:-1]):
                nop = nc.engines[ins.engine].nop(nofuse=True, hint="waitsplit")
                ni = nop.ins
                cur = nc.cur_bb.bb.instructions
                if ni in cur:
                    cur.remove(ni)
                ni.sync_info = SyncInfo(on_wait=[w], on_update=[])
                insts.insert(idx + j, ni)
                n += 1
            ins.sync_info = SyncInfo(on_wait=[waits[-1]], on_update=si.on_update)
    return n


def r3(ap, e=E):
    return ap.rearrange("p (t e) -> p t e", e=e)


def build():
    nc = bacc.Bacc("TRN2", target_bir_lowering=False, debug=False,
                   num_devices=N_CORES)

    x_ap = nc.dram_tensor("x", [S, M], f32, kind="ExternalInput").ap()
    xts_ap = nc.dram_tensor("xts", [M, S // N_CORES], f32, kind="ExternalInput").ap()
    wg_ap = nc.dram_tensor("wg", [M, E], f32, kind="ExternalInput").ap()
    w1_ap = nc.dram_tensor("w1", [M, H], f16, kind="ExternalInput").ap()
    b1c_ap = nc.dram_tensor("b1c", [128, H // 128], f32, kind="ExternalInput").ap()
    w2t_ap = nc.dram_tensor("w2t", [M // 128, 128, H], f16, kind="ExternalInput").ap()
    b2c_ap = nc.dram_tensor("b2c", [128, M // 128], f32, kind="ExternalInput").ap()
    eid_ap = nc.dram_tensor("eid", [128, 1], f32, kind="ExternalInput").ap()

    eot_ap = nc.dram_tensor("EOT", [M, C], f32, kind="ExternalOutput").ap()
    a_ap = nc.dram_tensor("A", [C, 1], i32, kind="ExternalOutput").ap()
    g_ap = nc.dram_tensor("G", [C, 1], f32, kind="ExternalOutput").ap()

    with tile.TileContext(nc) as tc:
        with tc.tile_pool(name="consts", bufs=1) as cp, \
             tc.tile_pool(name="rt", bufs=1) as rt, \
             tc.tile_pool(name="dram", bufs=1, space="DRAM") as dram:

            # ---------------- constants ----------------
            TRI = cp.tile([128, 128], f32)       # TRI[q,p]=1 iff q<=p
            make_upper_triangular(nc, TRI[:], val=1.0, diag=True)
            TRIS = cp.tile([64, 64], f32)        # strict upper
            make_upper_triangular(nc, TRIS[:], val=1.0, diag=False)
            IDN = cp.tile([128, 128], f32)
            make_identity(nc, IDN[:])
            ones_col = cp.tile([128, 1], f32)
            nc.vector.memset(ones_col[:], 1.0)
            ones_row = cp.tile([1, 128], f32)
            nc.vector.memset(ones_row[:], 1.0)
            iota_e = cp.tile([128, NT * E], f32)
            nc.gpsimd.iota(r3(iota_e[:]), pattern=[[0, NT], [1, E]], base=0,
                           channel_multiplier=0,
                           allow_small_or_imprecise_dtypes=True)
            tokf = cp.tile([128, NT], f32)
            nc.gpsimd.iota(tokf[:], pattern=[[128, NT]], base=0,
                           channel_multiplier=1,
                           allow_small_or_imprecise_dtypes=True)
            eid_sb = cp.tile([128, 1], f32)
            nc.sync.dma_start(eid_sb[:], eid_ap[:])
            b1c = cp.tile([128, H // 128], f32)
            nc.sync.dma_start(b1c[:], b1c_ap[:])
            b2c = cp.tile([128, M // 128], f32)
            nc.sync.dma_start(b2c[:], b2c_ap[:])

            # preload the sparse_gather ucode library while the PE is busy
            dumv = cp.tile([16, 8], f32)
            nc.vector.memset(dumv[:], 1.0)
            dumo = cp.tile([16, 8], f32)
            dumn = cp.tile([1, 1], u32)
            nc.gpsimd.sparse_gather(dumo[:], dumv[:], num_found=dumn[:])
            zoff = cp.tile([128, 1], i32)
            nc.vector.memset(zoff[:], 0)

            # ---------------- phase 1: gate GEMM (fp32) ----------------
            wg_sb = cp.tile([128, (M // 128) * E], f32)   # [p, (mc, e)]
            nc.sync.dma_start(r3(wg_sb[:]),
                              wg_ap.rearrange("(mc p) e -> p mc e", p=128))
            Lg_sb = rt.tile([128, NTS * E], f32)          # own shard logits
            Lg_in = dram.tile([S // N_CORES, E], f32)
            Lg_all = dram.tile([S, E], f32, addr_space="Shared")

            with tc.tile_pool(name="xts", bufs=1) as xp, \
                 tc.tile_pool(name="psg_p", bufs=1, space="PSUM") as psg_p:
                psgs = [psg_p.tile([128, E], f32, tag=f"psg{t}", name=f"psg{t}")
                        for t in range(NTS)]
                for mc in range(M // 128):
                    st = xp.tile([128, S // N_CORES], f32, tag=f"xts{mc}")
                    nc.sync.dma_start(st[:], xts_ap[mc * 128:(mc + 1) * 128, :])
                    for t in range(NTS):
                        nc.tensor.matmul(
                            psgs[t][:], st[:, t * 128:(t + 1) * 128],
                            wg_sb[:, mc * E:(mc + 1) * E],
                            start=(mc == 0), stop=(mc == M // 128 - 1))
                for t in range(NTS):
                    nc.vector.tensor_copy(Lg_sb[:, t * E:(t + 1) * E], psgs[t][:])

            nc.sync.dma_start(Lg_in[:].rearrange("(t p) e -> p t e", p=128),
                              r3(Lg_sb[:]))
            nc.gpsimd.collective_compute(
                "AllGather", OP.bypass,
                replica_groups=[list(range(N_CORES))],
                ins=[Lg_in[:]], outs=[Lg_all[:]])

            bigp_cm = tc.tile_pool(name="big", bufs=1)
            bigp = bigp_cm.__enter__()
            wk_cm = tc.tile_pool(name="wk", bufs=2)
            wk = wk_cm.__enter__()

            # PE warm-up machinery: dummy matmuls on constant data keep the
            # HAM activity monitor at full clock through the DMA-heavy
            # prologue so the FFN GEMMs start warm.
            warm_cm = tc.tile_pool(name="warm_ps", bufs=1, space="PSUM")
            warm_ps = warm_cm.__enter__()
            warm_out = warm_ps.tile([128, 256], f32, tag="warm")
            warm_in = cp.tile([128, 256], mybir.dt.bfloat16)
            nc.vector.memset(warm_in[:], 1.0)
            warm_w = cp.tile([128, 128], mybir.dt.bfloat16)
            nc.vector.memset(warm_w[:], 1.0)

            def pe_warm(n):
                for _ in range(n):
                    nc.tensor.matmul(warm_out[:], warm_w[:], warm_in[:],
                                     start=True, stop=True,
                                     skip_group_check=True)

            # ---------------- phase 2: routing (replicated) ----------------
            L_all = rt.tile([128, NT * E], f32)
            for lc in range(8):
                tchunk = NT // 8
                nc.sync.dma_start(
                    r3(L_all[:, lc * tchunk * E:(lc + 1) * tchunk * E]),
                    Lg_all[lc * tchunk * 128:(lc + 1) * tchunk * 128, :]
                    .rearrange("(t p) e -> p t e", p=128))
            L3 = r3(L_all[:])

            lmax = rt.tile([128, NT], f32)
            nc.vector.tensor_reduce(lmax[:], L3, axis=AX.X, op=OP.max)
            lmax_b = lmax[:].rearrange("p (t o) -> p t o", o=1) \
                            .to_broadcast([128, NT, E])
            dd = rt.tile([128, NT * E], f32)
            nc.vector.tensor_tensor(out=r3(dd[:]), in0=L3, in1=lmax_b,
                                    op=OP.subtract)
            expd = rt.tile([128, NT * E], f32)
            nc.scalar.activation(expd[:], dd[:], ACTF.Exp)
            ssum = rt.tile([128, NT], f32)
            nc.vector.tensor_reduce(ssum[:], r3(expd[:]), axis=AX.X, op=OP.add)
            gv = rt.tile([128, NT], f32)
            nc.vector.reciprocal(gv[:], ssum[:])

            oh = rt.tile([128, NT * E], f32)
            nc.vector.tensor_tensor(out=r3(oh[:]), in0=L3, in1=lmax_b,
                                    op=OP.is_equal)
            tmp = rt.tile([128, NT * E], f32)
            nc.vector.tensor_mul(tmp[:], oh[:], iota_e[:])
            eidx = rt.tile([128, NT], f32)
            nc.vector.tensor_reduce(eidx[:], r3(tmp[:]), axis=AX.X, op=OP.add)

            ps_rt_cm = tc.tile_pool(name="ps_rt", bufs=1, space="PSUM")
            ps_rt = ps_rt_cm.__enter__()
            pos_ps = ps_rt.tile([128, NT * E], f32, tag="pos")
            nc.tensor.matmul(pos_ps[:], TRI[:], oh[:], start=True, stop=False)
            cnt_ps = ps_rt.tile([1, NT * E], f32, tag="cnt")
            nc.tensor.matmul(cnt_ps[:], ones_col[:], oh[:], start=True, stop=True)
            cnt_sb = rt.tile([1, NT * E], f32)
            nc.vector.tensor_copy(cnt_sb[:], cnt_ps[:])
            cntd = dram.tile([64, 8], f32)
            nc.sync.dma_start(cntd[:].rearrange("t e -> (t e)"), cnt_sb[0:1, :])
            cnt64 = rt.tile([64, 8], f32)
            nc.sync.dma_start(cnt64[:], cntd[:])
            offs_ps = ps_rt.tile([64, 8], f32, tag="offs")
            nc.tensor.matmul(offs_ps[:], TRIS[:], cnt64[:], start=True, stop=True)
            offs_sb = rt.tile([64, 8], f32)
            nc.vector.tensor_copy(offs_sb[:], offs_ps[:])
            offsd = dram.tile([64, 8], f32)
            nc.sync.dma_start(offsd[:], offs_sb[:])
            offs_flat = rt.tile([1, NT * E], f32)
            nc.sync.dma_start(offs_flat[0:1, :], offsd[:].rearrange("t e -> (t e)"))
            nc.tensor.matmul(pos_ps[:], ones_row[:], offs_flat[:],
                             start=False, stop=True)

            pe_warm(8)
            pm = rt.tile([128, NT * E], f32)
            nc.vector.tensor_mul(pm[:], pos_ps[:], oh[:])
            pos_tok = rt.tile([128, NT], f32)
            nc.vector.tensor_reduce(pos_tok[:], r3(pm[:]), axis=AX.X, op=OP.add)
            nc.vector.tensor_scalar_add(pos_tok[:], pos_tok[:], -1.0)

            keep = rt.tile([128, NT], f32)
            nc.vector.tensor_scalar(out=keep[:], in0=pos_tok[:],
                                    scalar1=float(C), scalar2=None, op0=OP.is_lt)
            mine = rt.tile([128, NT], f32)
            nc.vector.tensor_scalar(out=mine[:], in0=eidx[:],
                                    scalar1=eid_sb[:, 0:1], scalar2=None,
                                    op0=OP.is_equal)
            sel = rt.tile([128, NT], f32)
            nc.vector.tensor_mul(sel[:], mine[:], keep[:])
            gvk = rt.tile([128, NT], f32)
            nc.vector.tensor_mul(gvk[:], gv[:], keep[:])

            # masked payloads: value if sel else -1; cols NT..NT+7 stay 0
            # (always-kept sentinels completing the capacity C region)
            XCOL = NT + C // 128
            mtok = rt.tile([128, XCOL], f32)
            nc.vector.memset(mtok[:, NT:], 0.0)
            nc.vector.tensor_scalar_add(mtok[:, :NT], tokf[:], 1.0)
            nc.vector.tensor_mul(mtok[:, :NT], mtok[:, :NT], sel[:])
            nc.vector.tensor_scalar_add(mtok[:, :NT], mtok[:, :NT], -1.0)
            mgv = rt.tile([128, XCOL], f32)
            nc.vector.memset(mgv[:, NT:], 0.0)
            nc.vector.tensor_scalar_add(mgv[:, :NT], gvk[:], 1.0)
            nc.vector.tensor_mul(mgv[:, :NT], mgv[:, :NT], sel[:])
            nc.vector.tensor_scalar_add(mgv[:, :NT], mgv[:, :NT], -1.0)

            # ------ sparse_gather compaction: slot -> (tokid, gate) ------
            Vd1 = dram.tile([S + C], f32)
            Vd2 = dram.tile([S + C], f32)
            tps1 = ps_rt.tile([128, 128], f32, tag="vdt")
            nc.tensor.transpose(tps1[0:XCOL, :], mtok[:], IDN[:])
            mtokT = rt.tile([128, 128], f32)
            nc.vector.tensor_copy(mtokT[0:XCOL, :], tps1[0:XCOL, :])
            tps2 = ps_rt.tile([128, 128], f32, tag="vdt")
            nc.tensor.transpose(tps2[0:XCOL, :], mgv[:], IDN[:])
            mgvT = rt.tile([128, 128], f32)
            nc.vector.tensor_copy(mgvT[0:XCOL, :], tps2[0:XCOL, :])
            nc.sync.dma_start(Vd1[:].rearrange("(t p) -> t p", p=128),
                              mtokT[0:XCOL, :])
            nc.sync.dma_start(Vd2[:].rearrange("(t p) -> t p", p=128),
                              mgvT[0:XCOL, :])
            pe_warm(8)
            ps_rt_cm.__exit__(None, None, None)
            V1 = rt.tile([16, F_IN], f32)
            nc.sync.dma_start(V1[:], Vd1[:].rearrange("(f q) -> q f", q=16))
            V2 = rt.tile([16, F_IN], f32)
            nc.sync.dma_start(V2[:], Vd2[:].rearrange("(f q) -> q f", q=16))
            SG1 = rt.tile([16, F_OUT], f32)
            NF1 = rt.tile([1, 1], u32)
            nc.gpsimd.sparse_gather(SG1[:], V1[:], num_found=NF1[:])
            SG2 = rt.tile([16, F_OUT], f32)
            NF2 = rt.tile([1, 1], u32)
            nc.gpsimd.sparse_gather(SG2[:], V2[:], num_found=NF2[:])

            dumg = rt.tile([128, M], f32)
            nc.gpsimd.indirect_dma_start(
                out=dumg[:], out_offset=None, in_=x_ap,
                in_offset=bass.IndirectOffsetOnAxis(ap=zoff[:], axis=0))
            A_f = dram.tile([C], f32)
            nc.sync.dma_start(A_f[:].rearrange("(f q) -> q f", q=16), SG1[:])
            G_f = dram.tile([C], f32)
            nc.sync.dma_start(G_f[:].rearrange("(f q) -> q f", q=16), SG2[:])

            pe_warm(8)
            a_slot_f = rt.tile([128, C // 128], f32)
            nc.sync.dma_start(a_slot_f[:], A_f[:].rearrange("(s p) -> p s", p=128))
            a_i = rt.tile([128, C // 128], i32)
            nc.vector.tensor_copy(a_i[:], a_slot_f[:])
            nc.sync.dma_start(a_ap.rearrange("(s p) o -> p (s o)", p=128), a_i[:])
            g_sb = rt.tile([128, C // 128], f32)
            nc.sync.dma_start(g_sb[:], G_f[:].rearrange("(s p) -> p s", p=128))
            nc.sync.dma_start(g_ap.rearrange("(s p) o -> p (s o)", p=128), g_sb[:])

            # ---------------- phase 3: dispatch gather + transpose --------
            pe_warm(8)
            dispTs = [bigp.tile([128, (M // 128) * CBLK], f16, name=f"dispT{c}")
                      for c in range(NCB)]  # [p, (mc, c_in_blk)]
            ps_tr_cm = tc.tile_pool(name="ps_tr", bufs=2, space="PSUM")
            ps_tr = ps_tr_cm.__enter__()
            for sg in range(C // 128):
                dg = wk.tile([128, M], f32, tag="dg")
                nc.gpsimd.indirect_dma_start(
                    out=dg[:], out_offset=None, in_=x_ap,
                    in_offset=bass.IndirectOffsetOnAxis(
                        ap=a_i[:, sg:sg + 1], axis=0))
                for mc in range(M // 128):
                    tp = ps_tr.tile([128, 128], f32, tag="tr")
                    nc.tensor.transpose(tp[:], dg[:, mc * 128:(mc + 1) * 128],
                                        IDN[:])
                    cbb, sgo = divmod(sg, CBLK // 128)
                    nc.vector.tensor_copy(
                        dispTs[cbb][:, mc * CBLK + sgo * 128:
                                    mc * CBLK + (sgo + 1) * 128],
                        tp[:])

            # ---------------- phases 4+5: expert FFN ----------------
            ps_tr_cm.__exit__(None, None, None)
            warm_cm.__exit__(None, None, None)
            ps_ffn_cm = tc.tile_pool(name="ps_ffn", bufs=1, space="PSUM")
            ps_ffn = ps_ffn_cm.__enter__()
            hT = bigp.tile([128, (H // 128) * CBLK], f16)  # [p, (ht, c)] 64KB/p
            for cb in range(NCB):
                # GEMM1: hT[h, c] = relu(w1.T @ dispT + b1)
                for htb in range(H // CBLK):              # 8 blocks of 4 ht
                    w1s = []
                    for mc in range(M // 128):
                        w1sl = wk.tile([128, CBLK], f16, tag="w1sl", bufs=6)
                        nc.sync.dma_start(
                            w1sl[:],
                            w1_ap[mc * 128:(mc + 1) * 128,
                                  htb * CBLK:(htb + 1) * CBLK])
                        w1s.append(w1sl)
                    for hi in range(CBLK // 128):         # 4 ht per block
                        ht_i = htb * (CBLK // 128) + hi
                        ps1 = ps_ffn.tile([128, CBLK], f32, tag="g1", bufs=4)
                        for mc in range(M // 128):
                            nc.tensor.matmul(
                                ps1[:],
                                w1s[mc][:, hi * 128:(hi + 1) * 128],
                                dispTs[cb][:, mc * CBLK:(mc + 1) * CBLK],
                                start=(mc == 0), stop=(mc == M // 128 - 1))
                        nc.scalar.activation(
                            hT[:, ht_i * CBLK:(ht_i + 1) * CBLK], ps1[:],
                            ACTF.Relu, bias=b1c[:, ht_i:ht_i + 1], scale=1.0)

                # gate row broadcast for this c-block
                g_row = wk.tile([1, CBLK], f32, tag="grow")
                nc.sync.dma_start(g_row[0:1, :],
                                  G_f[cb * CBLK:(cb + 1) * CBLK])
                gb_ps = ps_ffn.tile([128, CBLK], f32, tag="g1", bufs=4)
                nc.tensor.matmul(gb_ps[:], ones_row[:], g_row[:],
                                 start=True, stop=True)
                g_bc = wk.tile([128, CBLK], f32, tag="gbc")
                nc.vector.tensor_copy(g_bc[:], gb_ps[:])

                # GEMM2: eoT[m, c] = w2.T @ hT ; then (+b2) * gate
                for mt in range(M // 128):
                    w2ts = wk.tile([128, H], f16, tag="w2ts", bufs=2)
                    nc.sync.dma_start(
                        w2ts[:], w2t_ap[mt].rearrange("p (hc m) -> p hc m", m=128))
                    ps2 = ps_ffn.tile([128, CBLK], f32, tag="g2", bufs=2)
                    for hc in range(H // 128):
                        nc.tensor.matmul(
                            ps2[:], w2ts[:, hc * 128:(hc + 1) * 128],
                            hT[:, hc * CBLK:(hc + 1) * CBLK],
                            start=(hc == 0), stop=(hc == H // 128 - 1))
                    eo_sb = wk.tile([128, CBLK], f32, tag="eo")
                    nc.vector.tensor_scalar(out=eo_sb[:], in0=ps2[:],
                                            scalar1=b2c[:, mt:mt + 1],
                                            scalar2=None, op0=OP.add)
                    nc.vector.tensor_mul(eo_sb[:], eo_sb[:], g_bc[:])
                    nc.sync.dma_start(
                        eot_ap[mt * 128:(mt + 1) * 128,
                               cb * CBLK:(cb + 1) * CBLK], eo_sb[:])

            ps_ffn_cm.__exit__(None, None, None)
            wk_cm.__exit__(None, None, None)
            bigp_cm.__exit__(None, None, None)

    nc.compile()
    _split_multi_waits(nc)
    return nc


_NC_CACHE = None


def _get_nc():
    global _NC_CACHE
    if _NC_CACHE is None:
        _NC_CACHE = build()
    return _NC_CACHE


def _make_in_maps(x, wg, w1, b1, w2, b2):
    x2 = np.ascontiguousarray(np.asarray(x, np.float32).reshape(S, M))
    wg = np.ascontiguousarray(np.asarray(wg, np.float32))
    w1 = np.asarray(w1, np.float32)
    b1 = np.asarray(b1, np.float32)
    w2 = np.asarray(w2, np.float32)
    b2 = np.asarray(b2, np.float32)
    in_maps = []
    for k in range(N_CORES):
        shard = x2[k * (S // N_CORES):(k + 1) * (S // N_CORES)]
        xts = np.ascontiguousarray(shard.T)                    # [M, S/8]
        w1k = np.ascontiguousarray(w1[k]).astype(np.float16)   # [M, H]
        b1ck = np.ascontiguousarray(b1[k].reshape(H // 128, 128).T)
        w2k = w2[k]                                            # [H, M]
        w2t = np.ascontiguousarray(
            w2k.reshape(H // 128, 128, M // 128, 128).transpose(2, 1, 0, 3)
        ).astype(np.float16)
        b2ck = np.ascontiguousarray(b2[k].reshape(M // 128, 128).T)
        eid = np.full((128, 1), k, np.float32)
        in_maps.append({
            "x": x2, "xts": xts, "wg": wg, "w1": w1k, "b1c": b1ck,
            "w2t": w2t, "b2c": b2ck, "eid": eid,
        })
    return in_maps


def run_cores(x, wg, w1, b1, w2, b2, trace=False, tmpdir=None):
    nc = _get_nc()
    in_maps = _make_in_maps(x, wg, w1, b1, w2, b2)
    return run_bass_kernel_spmd(nc, in_maps, list(range(N_CORES)), trace=trace,
                                tmpdir=tmpdir)


def combine(results):
    out = np.zeros((S, M), np.float32)
    for k in range(N_CORES):
        r = results[k]
        eo = np.ascontiguousarray(r["EOT"].T)    # [C, M]
        A = r["A"][:, 0].astype(np.int64)
        G = r["G"][:, 0]
        valid = G > 0
        out[A[valid]] = eo[valid]
    return out.reshape(B, SQ, M)


def kernel(x, wg, w1, b1, w2, b2):
    res = run_cores(x, wg, w1, b1, w2, b2, trace=False)
    return combine(res.results)


# revision 15
# speedup vs baseline: 1.1771x; 1.0279x over previous
"""MoE (top-1, capacity_factor=1) Trainium2 Bass kernel, expert-parallel over
8 NeuronCores. Self-contained: imports only numpy + concourse (/opt).

Per-core k (expert k resident):
  gate GEMM (fp32 exact) on its 1/8 token shard -> AllGather logits ->
  replicated routing (one-hot, global cumsum via triangular matmuls,
  capacity mask) -> slot->token table via gpsimd sparse_gather ->
  indirect row-gather of x + PE transposes -> dispT [m, c] ->
  GEMM1 (fp32r, w1 stationary) -> hT [h, c] with fused ReLU+b1 ->
  GEMM2 (fp32r, w2 stationary, host pre-tiled) -> eoT [m, c],
  + b2 + gate scaling -> EOT output.
Host: scatter rows eo = EOT.T back by token id (A table), empties G==0.
"""
import sys

sys.path.insert(0, '/opt/trn_rl_repo')

import numpy as np
import concourse.bass as bass
import concourse.tile as tile
import concourse.mybir as mybir
from concourse import bacc
from concourse.bass_utils import run_bass_kernel_spmd
from concourse.masks import make_upper_triangular, make_identity

N_CORES = 8
B, SQ, M, E, H = 4, 2048, 1024, 8, 4096
S = B * SQ            # 8192 tokens
C = S // E            # 1024 capacity
NT = S // 128         # 64 token tiles
NTS = NT // N_CORES   # 8 token tiles per core shard
CBLK = 512            # c-block for GEMM1/GEMM2
NCB = C // CBLK       # 2 c-blocks
BIG = 2_000_000.0
F_IN = (S + C) // 16  # 576  sparse_gather input free size
F_OUT = C // 16       # 64

f32 = mybir.dt.float32
f32r = mybir.dt.float32r
f16 = mybir.dt.float16
i32 = mybir.dt.int32
u32 = mybir.dt.uint32
AX = mybir.AxisListType
OP = mybir.AluOpType
ACTF = mybir.ActivationFunctionType


def _split_multi_waits(nc):
    """This walrus build accepts at most ONE sync-wait per instruction.
    Split extras into same-engine NOPs inserted just before."""
    from concourse.mybir import SyncInfo
    n = 0
    for bb in list(nc.main_func.blocks):
        insts = bb.instructions  # live shared list
        for ins in list(insts):
            si = ins.sync_info
            if si is None or len(si.on_wait) <= 1:
                continue
            waits = list(si.on_wait)
            idx = insts.index(ins)
            for j, w in enumerate(waits[:-1]):
                nop = nc.engines[ins.engine].nop(nofuse=True, hint="waitsplit")
                ni = nop.ins
                cur = nc.cur_bb.bb.instructions
                if ni in cur:
                    cur.remove(ni)
                ni.sync_info = SyncInfo(on_wait=[w], on_update=[])
                insts.insert(idx + j, ni)
                n += 1
            ins.sync_info = SyncInfo(on_wait=[waits[-1]], on_update=si.on_update)
    return n


def r3(ap, e=E):
    return ap.rearrange("p (t e) -> p t e", e=e)


def build():
    nc = bacc.Bacc("TRN2", target_bir_lowering=False, debug=False,
                   num_devices=N_CORES)

    x_ap = nc.dram_tensor("x", [S, M], f32, kind="ExternalInput").ap()
    xts_ap = nc.dram_tensor("xts", [M, S // N_CORES], f32, kind="ExternalInput").ap()
    wg_ap = nc.dram_tensor("wg", [M, E], f32, kind="ExternalInput").ap()
    w1_ap = nc.dram_tensor("w1", [M, H], f16, kind="ExternalInput").ap()
    b1c_ap = nc.dram_tensor("b1c", [128, H // 128], f32, kind="ExternalInput").ap()
    w2t_ap = nc.dram_tensor("w2t", [M // 128, 128, H], f16, kind="ExternalInput").ap()
    b2c_ap = nc.dram_tensor("b2c", [128, M // 128], f32, kind="ExternalInput").ap()
    eid_ap = nc.dram_tensor("eid", [128, 1], f32, kind="ExternalInput").ap()

    eot_ap = nc.dram_tensor("EOT", [M, C], f32, kind="ExternalOutput").ap()
    a_ap = nc.dram_tensor("A", [C, 1], i32, kind="ExternalOutput").ap()
    g_ap = nc.dram_tensor("G", [C, 1], f32, kind="ExternalOutput").ap()

    with tile.TileContext(nc) as tc:
        with tc.tile_pool(name="consts", bufs=1) as cp, \
             tc.tile_pool(name="rt", bufs=1) as rt, \
             tc.tile_pool(name="dram", bufs=1, space="DRAM") as dram:

            # ---------------- constants ----------------
            TRI = cp.tile([128, 128], f32)       # TRI[q,p]=1 iff q<=p
            make_upper_triangular(nc, TRI[:], val=1.0, diag=True)
            TRIS = cp.tile([64, 64], f32)        # strict upper
            make_upper_triangular(nc, TRIS[:], val=1.0, diag=False)
            IDN = cp.tile([128, 128], f32)
            make_identity(nc, IDN[:])
            ones_col = cp.tile([128, 1], f32)
            nc.vector.memset(ones_col[:], 1.0)
            ones_row = cp.tile([1, 128], f32)
            nc.vector.memset(ones_row[:], 1.0)
            iota_e = cp.tile([128, NT * E], f32)
            nc.gpsimd.iota(r3(iota_e[:]), pattern=[[0, NT], [1, E]], base=0,
                           channel_multiplier=0,
                           allow_small_or_imprecise_dtypes=True)
            tokf = cp.tile([128, NT], f32)
            nc.gpsimd.iota(tokf[:], pattern=[[128, NT]], base=0,
                           channel_multiplier=1,
                           allow_small_or_imprecise_dtypes=True)
            eid_sb = cp.tile([128, 1], f32)
            nc.sync.dma_start(eid_sb[:], eid_ap[:])
            b1c = cp.tile([128, H // 128], f32)
            nc.sync.dma_start(b1c[:], b1c_ap[:])
            b2c = cp.tile([128, M // 128], f32)
            nc.sync.dma_start(b2c[:], b2c_ap[:])

            # preload the sparse_gather ucode library while the PE is busy
            dumv = cp.tile([16, 8], f32)
            nc.vector.memset(dumv[:], 1.0)
            dumo = cp.tile([16, 8], f32)
            dumn = cp.tile([1, 1], u32)
            nc.gpsimd.sparse_gather(dumo[:], dumv[:], num_found=dumn[:])
            zoff = cp.tile([128, 1], i32)
            nc.vector.memset(zoff[:], 0)
            dumg = cp.tile([128, M], f32)
            nc.gpsimd.indirect_dma_start(
                out=dumg[:], out_offset=None, in_=x_ap,
                in_offset=bass.IndirectOffsetOnAxis(ap=zoff[:], axis=0))

            # ---------------- phase 1: gate GEMM (fp32) ----------------
            wg_sb = cp.tile([128, (M // 128) * E], f32)   # [p, (mc, e)]
            nc.sync.dma_start(r3(wg_sb[:]),
                              wg_ap.rearrange("(mc p) e -> p mc e", p=128))
            Lg_sb = rt.tile([128, NTS * E], f32)          # own shard logits
            Lg_in = dram.tile([S // N_CORES, E], f32)
            Lg_all = dram.tile([S, E], f32, addr_space="Shared")

            with tc.tile_pool(name="xts", bufs=1) as xp, \
                 tc.tile_pool(name="psg_p", bufs=1, space="PSUM") as psg_p:
                psgs = [psg_p.tile([128, E], f32, tag=f"psg{t}", name=f"psg{t}")
                        for t in range(NTS)]
                for mc in range(M // 128):
                    st = xp.tile([128, S // N_CORES], f32, tag=f"xts{mc}")
                    nc.sync.dma_start(st[:], xts_ap[mc * 128:(mc + 1) * 128, :])
                    for t in range(NTS):
                        nc.tensor.matmul(
                            psgs[t][:], st[:, t * 128:(t + 1) * 128],
                            wg_sb[:, mc * E:(mc + 1) * E],
                            start=(mc == 0), stop=(mc == M // 128 - 1))
                for t in range(NTS):
                    nc.vector.tensor_copy(Lg_sb[:, t * E:(t + 1) * E], psgs[t][:])

            nc.sync.dma_start(Lg_in[:].rearrange("(t p) e -> p t e", p=128),
                              r3(Lg_sb[:]))
            nc.gpsimd.collective_compute(
                "AllGather", OP.bypass,
                replica_groups=[list(range(N_CORES))],
                ins=[Lg_in[:]], outs=[Lg_all[:]])

            bigp_cm = tc.tile_pool(name="big", bufs=1)
            bigp = bigp_cm.__enter__()
            wk_cm = tc.tile_pool(name="wk", bufs=2)
            wk = wk_cm.__enter__()

            # PE warm-up machinery: dummy matmuls on constant data keep the
            # HAM activity monitor at full clock through the DMA-heavy
            # prologue so the FFN GEMMs start warm.
            warm_cm = tc.tile_pool(name="warm_ps", bufs=1, space="PSUM")
            warm_ps = warm_cm.__enter__()
            warm_out = warm_ps.tile([128, 256], f32, tag="warm")
            warm_in = cp.tile([128, 256], mybir.dt.bfloat16)
            nc.vector.memset(warm_in[:], 1.0)
            warm_w = cp.tile([128, 128], mybir.dt.bfloat16)
            nc.vector.memset(warm_w[:], 1.0)

            def pe_warm(n):
                for _ in range(n):
                    nc.tensor.matmul(warm_out[:], warm_w[:], warm_in[:],
                                     start=True, stop=True,
                                     skip_group_check=True)

            # ---------------- phase 2: routing (replicated) ----------------
            L_all = rt.tile([128, NT * E], f32)
            for lc in range(8):
                tchunk = NT // 8
                nc.sync.dma_start(
                    r3(L_all[:, lc * tchunk * E:(lc + 1) * tchunk * E]),
                    Lg_all[lc * tchunk * 128:(lc + 1) * tchunk * 128, :]
                    .rearrange("(t p) e -> p t e", p=128))
            L3 = r3(L_all[:])

            lmax = rt.tile([128, NT], f32)
            nc.vector.tensor_reduce(lmax[:], L3, axis=AX.X, op=OP.max)
            lmax_b = lmax[:].rearrange("p (t o) -> p t o", o=1) \
                            .to_broadcast([128, NT, E])
            dd = rt.tile([128, NT * E], f32)
            nc.vector.tensor_tensor(out=r3(dd[:]), in0=L3, in1=lmax_b,
                                    op=OP.subtract)
            expd = rt.tile([128, NT * E], f32)
            nc.scalar.activation(expd[:], dd[:], ACTF.Exp)
            ssum = rt.tile([128, NT], f32)
            nc.vector.tensor_reduce(ssum[:], r3(expd[:]), axis=AX.X, op=OP.add)
            gv = rt.tile([128, NT], f32)
            nc.vector.reciprocal(gv[:], ssum[:])

            oh = rt.tile([128, NT * E], f32)
            nc.vector.tensor_tensor(out=r3(oh[:]), in0=L3, in1=lmax_b,
                                    op=OP.is_equal)
            tmp = rt.tile([128, NT * E], f32)
            nc.vector.tensor_mul(tmp[:], oh[:], iota_e[:])
            eidx = rt.tile([128, NT], f32)
            nc.vector.tensor_reduce(eidx[:], r3(tmp[:]), axis=AX.X, op=OP.add)

            ps_rt_cm = tc.tile_pool(name="ps_rt", bufs=1, space="PSUM")
            ps_rt = ps_rt_cm.__enter__()
            pos_ps = ps_rt.tile([128, NT * E], f32, tag="pos")
            nc.tensor.matmul(pos_ps[:], TRI[:], oh[:], start=True, stop=False)
            cnt_ps = ps_rt.tile([64, 8], f32, tag="cnt")
            oh3 = oh[:].rearrange("p (t e) -> p e t", e=E)
            for e in range(E):
                nc.tensor.matmul(cnt_ps[:, e:e + 1], oh3[:, e, :], ones_col[:],
                                 start=True, stop=True)
            cnt64 = rt.tile([64, 8], f32)
            nc.vector.tensor_copy(cnt64[:], cnt_ps[:])
            offs_ps = ps_rt.tile([64, 8], f32, tag="offs")
            nc.tensor.matmul(offs_ps[:], TRIS[:], cnt64[:], start=True, stop=True)
            offs_sb = rt.tile([64, 8], f32)
            nc.vector.tensor_copy(offs_sb[:], offs_ps[:])
            offsd = dram.tile([64, 8], f32)
            nc.sync.dma_start(offsd[:], offs_sb[:])
            offs_flat = rt.tile([1, NT * E], f32)
            nc.sync.dma_start(offs_flat[0:1, :], offsd[:].rearrange("t e -> (t e)"))
            nc.tensor.matmul(pos_ps[:], ones_row[:], offs_flat[:],
                             start=False, stop=True)

            pe_warm(8)
            pm = rt.tile([128, NT * E], f32)
            nc.vector.tensor_mul(pm[:], pos_ps[:], oh[:])
            pos_tok = rt.tile([128, NT], f32)
            nc.vector.tensor_reduce(pos_tok[:], r3(pm[:]), axis=AX.X, op=OP.add)
            nc.vector.tensor_scalar_add(pos_tok[:], pos_tok[:], -1.0)

            keep = rt.tile([128, NT], f32)
            nc.vector.tensor_scalar(out=keep[:], in0=pos_tok[:],
                                    scalar1=float(C), scalar2=None, op0=OP.is_lt)
            mine = rt.tile([128, NT], f32)
            nc.vector.tensor_scalar(out=mine[:], in0=eidx[:],
                                    scalar1=eid_sb[:, 0:1], scalar2=None,
                                    op0=OP.is_equal)
            sel = rt.tile([128, NT], f32)
            nc.vector.tensor_mul(sel[:], mine[:], keep[:])
            gvk = rt.tile([128, NT], f32)
            nc.vector.tensor_mul(gvk[:], gv[:], keep[:])

            # packed payload: tokid*2048 + round(gv*2047) if sel else -1;
            # cols NT..NT+7 stay 0 (always-kept sentinels -> A=0, G=0)
            XCOL = NT + C // 128
            mtok = rt.tile([128, XCOL], f32)
            nc.vector.memset(mtok[:, NT:], 0.0)
            nc.vector.tensor_scalar(out=mtok[:, :NT], in0=tokf[:], scalar1=2048.0,
                                    scalar2=1.0, op0=OP.mult, op1=OP.add)
            gq = rt.tile([128, NT], f32)
            nc.vector.tensor_scalar_mul(gq[:], gvk[:], 2047.0)
            nc.vector.tensor_add(mtok[:, :NT], mtok[:, :NT], gq[:])
            nc.vector.tensor_mul(mtok[:, :NT], mtok[:, :NT], sel[:])
            nc.vector.tensor_scalar_add(mtok[:, :NT], mtok[:, :NT], -1.0)

            # ------ sparse_gather compaction: slot -> (tokid, gate) ------
            Vd1 = dram.tile([S + C], f32)
            tps1 = ps_rt.tile([128, 128], f32, tag="vdt")
            nc.tensor.transpose(tps1[0:XCOL, :], mtok[:], IDN[:])
            mtokT = rt.tile([128, 128], f32)
            nc.vector.tensor_copy(mtokT[0:XCOL, :], tps1[0:XCOL, :])
            nc.sync.dma_start(Vd1[:].rearrange("(t p) -> t p", p=128),
                              mtokT[0:XCOL, :])
            pe_warm(8)
            ps_rt_cm.__exit__(None, None, None)
            V1 = rt.tile([16, F_IN], f32)
            nc.sync.dma_start(V1[:], Vd1[:].rearrange("(f q) -> q f", q=16))
            SG1 = rt.tile([16, F_OUT], f32)
            NF1 = rt.tile([1, 1], u32)
            nc.gpsimd.sparse_gather(SG1[:], V1[:], num_found=NF1[:])

            A_f = dram.tile([C], f32)
            nc.sync.dma_start(A_f[:].rearrange("(f q) -> q f", q=16), SG1[:])

            pe_warm(8)
            p_slot = rt.tile([128, C // 128], f32)
            nc.sync.dma_start(p_slot[:], A_f[:].rearrange("(s p) -> p s", p=128))
            p_i = rt.tile([128, C // 128], i32)
            nc.vector.tensor_copy(p_i[:], p_slot[:])
            a_i = rt.tile([128, C // 128], i32)
            nc.vector.tensor_scalar(out=a_i[:], in0=p_i[:], scalar1=11,
                                    scalar2=None, op0=OP.arith_shift_right)
            nc.sync.dma_start(a_ap.rearrange("(s p) o -> p (s o)", p=128), a_i[:])
            gq_i = rt.tile([128, C // 128], i32)
            nc.vector.tensor_scalar(out=gq_i[:], in0=p_i[:], scalar1=2047,
                                    scalar2=None, op0=OP.bitwise_and)
            g_sb = rt.tile([128, C // 128], f32)
            nc.vector.tensor_copy(g_sb[:], gq_i[:])
            nc.vector.tensor_scalar_mul(g_sb[:], g_sb[:], 1.0 / 2047.0)
            nc.sync.dma_start(g_ap.rearrange("(s p) o -> p (s o)", p=128), g_sb[:])
            G_f = dram.tile([C], f32)
            nc.sync.dma_start(G_f[:].rearrange("(s p) -> p s", p=128), g_sb[:])

            # ---------------- phase 3: dispatch gather + transpose --------
            pe_warm(8)
            dispTs = [bigp.tile([128, (M // 128) * CBLK], f16, name=f"dispT{c}")
                      for c in range(NCB)]  # [p, (mc, c_in_blk)]
            ps_tr_cm = tc.tile_pool(name="ps_tr", bufs=2, space="PSUM")
            ps_tr = ps_tr_cm.__enter__()
            for sg in range(C // 128):
                dg = wk.tile([128, M], f32, tag="dg", bufs=3)
                nc.gpsimd.indirect_dma_start(
                    out=dg[:], out_offset=None, in_=x_ap,
                    in_offset=bass.IndirectOffsetOnAxis(
                        ap=a_i[:, sg:sg + 1], axis=0))
                for mc in range(M // 128):
                    tp = ps_tr.tile([128, 128], f32, tag="tr")
                    nc.tensor.transpose(tp[:], dg[:, mc * 128:(mc + 1) * 128],
                                        IDN[:])
                    cbb, sgo = divmod(sg, CBLK // 128)
                    nc.vector.tensor_copy(
                        dispTs[cbb][:, mc * CBLK + sgo * 128:
                                    mc * CBLK + (sgo + 1) * 128],
                        tp[:])

            # ---------------- phases 4+5: expert FFN ----------------
            ps_tr_cm.__exit__(None, None, None)
            warm_cm.__exit__(None, None, None)
            ps_ffn_cm = tc.tile_pool(name="ps_ffn", bufs=1, space="PSUM")
            ps_ffn = ps_ffn_cm.__enter__()
            hTs = [bigp.tile([128, (H // 128) * CBLK], f16, name=f"hT{c}")
                   for c in range(NCB)]
            for cb in range(NCB):
                hT = hTs[cb]
                # GEMM1: hT[h, c] = relu(w1.T @ dispT + b1)
                for htb in range(H // CBLK):              # 8 blocks of 4 ht
                    w1s = []
                    for mc in range(M // 128):
                        w1sl = wk.tile([128, CBLK], f16, tag="w1sl", bufs=6)
                        nc.sync.dma_start(
                            w1sl[:],
                            w1_ap[mc * 128:(mc + 1) * 128,
                                  htb * CBLK:(htb + 1) * CBLK])
                        w1s.append(w1sl)
                    for hi in range(CBLK // 128):         # 4 ht per block
                        ht_i = htb * (CBLK // 128) + hi
                        ps1 = ps_ffn.tile([128, CBLK], f32, tag="g1", bufs=4)
                        for mc in range(M // 128):
                            nc.tensor.matmul(
                                ps1[:],
                                w1s[mc][:, hi * 128:(hi + 1) * 128],
                                dispTs[cb][:, mc * CBLK:(mc + 1) * CBLK],
                                start=(mc == 0), stop=(mc == M // 128 - 1))
                        nc.scalar.activation(
                            hT[:, ht_i * CBLK:(ht_i + 1) * CBLK], ps1[:],
                            ACTF.Relu, bias=b1c[:, ht_i:ht_i + 1], scale=1.0)

                # gate row broadcast for this c-block
                g_row = wk.tile([1, CBLK], f32, tag="grow")
                nc.sync.dma_start(g_row[0:1, :],
                                  G_f[cb * CBLK:(cb + 1) * CBLK])
                gb_ps = ps_ffn.tile([128, CBLK], f32, tag="g1", bufs=4)
                nc.tensor.matmul(gb_ps[:], ones_row[:], g_row[:],
                                 start=True, stop=True)
                g_bc = wk.tile([128, CBLK], f32, tag="gbc")
                nc.vector.tensor_copy(g_bc[:], gb_ps[:])

                # GEMM2: eoT[m, c] = w2.T @ hT ; then (+b2) * gate
                for mt in range(M // 128):
                    w2ts = wk.tile([128, H], f16, tag="w2ts", bufs=2)
                    nc.sync.dma_start(
                        w2ts[:], w2t_ap[mt].rearrange("p (hc m) -> p hc m", m=128))
                    ps2 = ps_ffn.tile([128, CBLK], f32, tag="g2", bufs=2)
                    for hc in range(H // 128):
                        nc.tensor.matmul(
                            ps2[:], w2ts[:, hc * 128:(hc + 1) * 128],
                            hT[:, hc * CBLK:(hc + 1) * CBLK],
                            start=(hc == 0), stop=(hc == H // 128 - 1))
                    eo_sb = wk.tile([128, CBLK], f32, tag="eo")
                    nc.vector.tensor_scalar(out=eo_sb[:], in0=ps2[:],
                                            scalar1=b2c[:, mt:mt + 1],
                                            scalar2=None, op0=OP.add)
                    nc.vector.tensor_mul(eo_sb[:], eo_sb[:], g_bc[:])
                    nc.sync.dma_start(
                        eot_ap[mt * 128:(mt + 1) * 128,
                               cb * CBLK:(cb + 1) * CBLK], eo_sb[:])

            ps_ffn_cm.__exit__(None, None, None)
            wk_cm.__exit__(None, None, None)
            bigp_cm.__exit__(None, None, None)

    nc.compile()
    _split_multi_waits(nc)
    return nc


_NC_CACHE = None


def _get_nc():
    global _NC_CACHE
    if _NC_CACHE is None:
        _NC_CACHE = build()
    return _NC_CACHE


def _make_in_maps(x, wg, w1, b1, w2, b2):
    x2 = np.ascontiguousarray(np.asarray(x, np.float32).reshape(S, M))
    wg = np.ascontiguousarray(np.asarray(wg, np.float32))
    w1 = np.asarray(w1, np.float32)
    b1 = np.asarray(b1, np.float32)
    w2 = np.asarray(w2, np.float32)
    b2 = np.asarray(b2, np.float32)
    in_maps = []
    for k in range(N_CORES):
        shard = x2[k * (S // N_CORES):(k + 1) * (S // N_CORES)]
        xts = np.ascontiguousarray(shard.T)                    # [M, S/8]
        w1k = np.ascontiguousarray(w1[k]).astype(np.float16)   # [M, H]
        b1ck = np.ascontiguousarray(b1[k].reshape(H // 128, 128).T)
        w2k = w2[k]                                            # [H, M]
        w2t = np.ascontiguousarray(
            w2k.reshape(H // 128, 128, M // 128, 128).transpose(2, 1, 0, 3)
        ).astype(np.float16)
        b2ck = np.ascontiguousarray(b2[k].reshape(M // 128, 128).T)
        eid = np.full((128, 1), k, np.float32)
        in_maps.append({
            "x": x2, "xts": xts, "wg": wg, "w1": w1k, "b1c": b1ck,
            "w2t": w2t, "b2c": b2ck, "eid": eid,
        })
    return in_maps


def run_cores(x, wg, w1, b1, w2, b2, trace=False, tmpdir=None):
    nc = _get_nc()
    in_maps = _make_in_maps(x, wg, w1, b1, w2, b2)
    return run_bass_kernel_spmd(nc, in_maps, list(range(N_CORES)), trace=trace,
                                tmpdir=tmpdir)


def combine(results):
    out = np.zeros((S, M), np.float32)
    for k in range(N_CORES):
        r = results[k]
        eo = np.ascontiguousarray(r["EOT"].T)    # [C, M]
        A = r["A"][:, 0].astype(np.int64)
        G = r["G"][:, 0]
        valid = G > 0
        out[A[valid]] = eo[valid]
    return out.reshape(B, SQ, M)


def kernel(x, wg, w1, b1, w2, b2):
    res = run_cores(x, wg, w1, b1, w2, b2, trace=False)
    return combine(res.results)
